# revision 6
# baseline (speedup 1.0000x reference)
"""Multi-head attention (RoPE + RMS-norm + structured mask bias) on 8 trn2
NeuronCores.

Sharding: B=4 batches x 2 half-head groups -> 8 cores. Core c handles batch
c//2 and heads 3*(c%2) .. 3*(c%2)+2. Each core computes per-head partial
outputs y_h @ Wproj_h.T summed over its 3 heads; the host adds the two
half-head partials per batch.

Math notes:
- bias = clip(g1*mi + g2*mj + g3*mi*mj, +-2) with mi,mj in {0,1} takes 4
  values; decomposed as a0 + a1*mi + a2*mj + a3*mi*mj. The a0 + a1*mi part
  is constant along the softmax axis (keys) and cancels; only
  (a2 + a3*mi)*mj survives. It is folded into the QK^T matmul as one extra
  contraction dim.
- The K-side RMS scale rkn = 8/sqrt(ssk) is NOT applied to k. Instead the
  scores matmul computes ps = (q_rms/8).k_raw + augq*augk with
  augq = a2+a3*mi, augk = mj*sqrt(ssk)/8, and the exp applies rkn as a
  per-partition (per-key) ACT scale: exp(rkn*ps) == exp(q.k/8 + bias).
  This deletes the k-half of the phase-A normalize multiply.
- scores are in [-9.5, 9.5], so softmax runs without max subtraction.
  The denominator comes free from an appended ones-column on v.
- All matmuls run in bf16; PSUM stays f32. PE streams 1 moving col/cycle,
  so phase B (2*T^2/128 columns per head) is the tensor-time floor.
- rms scales via exp/ln -> ONE ACT table set (natural_log_exp_and_others).
- ~1/3 of exp tiles run on the otherwise-idle DVE as a Schraudolph
  approximation (bits = round(A*rkn*x + B) as int16 into the bf16 tile);
  the per-key A*rkn lives in a per-partition scalar AP.
- HAM warmup: identity matmuls during the initial DMA wait keep the PE
  clock at 2.4GHz (otherwise all of phase A runs throttled at 1.2GHz).
- Rope cos/sin are expanded once on-device into dense per-head tables so
  the per-chunk rope multiplies use fast dense access patterns instead of
  broadcast APs. Rope intermediates are bf16 (2x DVE throughput).
- Per-head denominator transposes + reciprocal are emitted two j-steps
  into the NEXT head's matmul stream (borrowed scores-pool PSUM slots) so
  the PE FIFO never stalls on the py->yn copy.
- Phase C: projection matmuls for heads 0/1 stream first (covering head
  2's epilogue), then head 2's; combine = ACT scale + two pipelined DVE
  scalar_tensor_tensor ops into a resident f32 accumulator; output DMA'd
  per 4-chunk group.
"""

from contextlib import ExitStack

import numpy as np
import ml_dtypes

import concourse.bass as bass
import concourse.tile as tile
from concourse import mybir
from concourse.masks import make_identity
from concourse.bass_utils import run_bass_kernel_spmd
from concourse.vector_clock import ScopedClock
import concourse.tile as tile_mod

B, T, C, H, D = 4, 2048, 384, 6, 64
NHC = 3           # heads per core
TCN = T // 128    # 16 token chunks
EPS = float(np.finfo(np.float32).eps)
LN8 = float(np.log(8.0))

# Schraudolph exp in bf16 bit space: bits = round(A*x + B)
LOG2E = 1.4426950408889634
A_EXP = 128.0 * LOG2E
B_EXP = 127.0 * 128.0 - 5.6

f32 = mybir.dt.float32
bf16 = mybir.dt.bfloat16
i16 = mybir.dt.int16
i32 = mybir.dt.int32
OP = mybir.AluOpType
AF = mybir.ActivationFunctionType
AX = mybir.AxisListType

# ---------------------------------------------------------------------------
# Workaround: this container's walrus accepts at most ONE sync wait per
# instruction. Split the TileContext final drain, and post-process all
# instructions, hoisting extra waits onto same-engine NoOps.
# ---------------------------------------------------------------------------
_ctr = [0]


def _drain_and_barrier_split(self, tick_clock, wait_clock):
    nc = self.nc
    drain_inst = nc.sync.drain()
    wait_clock.add_sem_waits(
        drain_inst.ins, ScopedClock({None: tick_clock.global_clock})
    )
    mi = drain_inst.ins
    si = mi.sync_info
    if si is not None and len(si.on_wait) > 1:
        waits = list(si.on_wait)
        mi.sync_info = mybir.SyncInfo(on_wait=waits[:1], on_update=list(si.on_update))
        for i in range(1, len(waits)):
            extra = nc.sync.drain()
            extra.ins.sync_info = mybir.SyncInfo(on_wait=[waits[i]], on_update=[])
    nc.all_engine_barrier()
    assert self.sems is not None
    popped = nc._tile_sem_poison_stack.pop()
    assert popped is self._sem_poison
    nc.clear_and_free_semaphores(list(self.sems.allocated().values()))
    nc.all_engine_barrier()


tile_mod.TileContext._drain_and_barrier = _drain_and_barrier_split


def _split_multi_waits(nc):
    for f in nc.m.functions:
        for bb in f.blocks:
            insts = bb.instructions
            out = []
            changed = False
            for inst in insts:
                si = getattr(inst, "sync_info", None)
                if si is not None and si.on_wait and len(si.on_wait) > 1:
                    waits = list(si.on_wait)
                    for w in waits[:-1]:
                        _ctr[0] += 1
                        out.append(
                            mybir.InstNoOp(
                                name=f"WSPLIT-{_ctr[0]}",
                                engine=inst.engine,
                                ins=[],
                                outs=[],
                                sync_info=mybir.SyncInfo(on_wait=[w], on_update=[]),
                            )
                        )
                    inst.sync_info = mybir.SyncInfo(
                        on_wait=[waits[-1]], on_update=list(si.on_update)
                    )
                    changed = True
                out.append(inst)
            if changed:
                insts[:] = out


# ---------------------------------------------------------------------------
# Device program (SPMD: same program on all 8 cores, data differs)
# ---------------------------------------------------------------------------


def _build_nc():
    nc = bass.Bass()
    # all tables partition-major so every DMA line is one contiguous burst
    xT = nc.declare_dram_parameter("xT", [C, T], bf16, isOutput=False)
    wqkv = nc.declare_dram_parameter("wqkv", [C, 576], bf16, isOutput=False)
    wpt = nc.declare_dram_parameter("wpt", [NHC, D, C], bf16, isOutput=False)
    # rope table: [...] = [cos | sin | -sin] packed per token chunk
    ropet = nc.declare_dram_parameter("ropet", [128, TCN, 3, 32], f32,
                                      isOutput=False)
    micf = nc.declare_dram_parameter("micf", [128, TCN + 2 * NHC], f32,
                                     isOutput=False)
    out = nc.declare_dram_parameter("out", [T, C], f32, isOutput=True)

    with tile.TileContext(nc) as tc, ExitStack() as ctx:
        const = ctx.enter_context(tc.tile_pool(name="const", bufs=1))
        big = ctx.enter_context(tc.tile_pool(name="big", bufs=1))

        # ---- loads: small consts first so phase A's elementwise work can
        # start as soon as the first QKV chunk lands ----
        xt = big.tile([128, 3, T], bf16, tag="xt")
        wq = big.tile([128, 3, 576], bf16, tag="wq")
        xTr = xT[:].rearrange("(n p) t -> p n t", p=128)
        nc.sync.dma_start(out=wq[:], in_=wqkv[:].rearrange("(n p) d -> p n d", p=128))
        nc.sync.dma_start(out=xt[:, 0, 0:512], in_=xTr[:, 0, 0:512])
        nc.sync.dma_start(out=xt[:, 1, 0:512], in_=xTr[:, 1, 0:512])
        nc.sync.dma_start(out=xt[:, 2, 0:512], in_=xTr[:, 2, 0:512])
        rtb = const.tile([128, TCN, 3, 32], f32, tag="rtb")
        nc.sync.dma_start(out=rtb[:], in_=ropet[:])
        micf_t = const.tile([128, TCN + 2 * NHC], f32, tag="micf")
        nc.sync.dma_start(out=micf_t[:], in_=micf[:])
        mi_t = micf_t[:, 0:TCN].bitcast(i32)
        cf = micf_t[:, TCN : TCN + 2 * NHC]
        nc.sync.dma_start(out=xt[:, 0, 512:T], in_=xTr[:, 0, 512:T])
        nc.sync.dma_start(out=xt[:, 1, 512:T], in_=xTr[:, 1, 512:T])
        nc.sync.dma_start(out=xt[:, 2, 512:T], in_=xTr[:, 2, 512:T])
        wp = const.tile([D, NHC, C], bf16, tag="wp")
        nc.sync.dma_start(out=wp[:], in_=wpt[:].rearrange("h d c -> d h c"))

        ident = const.tile([128, 128], bf16, tag="ident")
        make_identity(nc, ident[:])

        # ---- HAM warmup: dense identity matmuls while the DMAs land, so
        # the PE clock gate opens (K=8/8) before the real work starts ----
        with tc.tile_pool(name="psW", bufs=2, space="PSUM") as psW:
            for i in range(44):
                pw = psW.tile([128, 128], f32, tag="pw")
                nc.tensor.matmul(pw[:], ident[:], ident[:], start=True,
                                 stop=True)

        # dense rope tables: cos/sin repeated per (head, half) so per-chunk
        # rope multiplies read fast dense bf16 instead of broadcast APs
        csF = const.tile([128, TCN, 6, 2, 32], bf16, tag="csF")
        snF = const.tile([128, TCN, 6, 2, 32], bf16, tag="snF")
        cs3 = rtb[:, :, 0, :]                    # [128, TCN, 32]
        sn3 = rtb[:, :, 1:3, :]                  # [128, TCN, 2, 32]
        nc.vector.tensor_copy(
            out=csF[:],
            in_=cs3[:, :, None, None, :].to_broadcast([128, TCN, 6, 2, 32]),
        )
        nc.gpsimd.tensor_copy(
            out=snF[:],
            in_=sn3[:, :, None, :, :].to_broadcast([128, TCN, 6, 2, 32]),
        )

        mf = const.tile([128, TCN], f32, tag="mf")
        nc.vector.tensor_copy(out=mf[:], in_=mi_t)
        mfb = const.tile([128, TCN], bf16, tag="mfb")
        nc.gpsimd.tensor_copy(out=mfb[:], in_=mf[:])
        onesb = const.tile([128, TCN], bf16, tag="onesb")
        nc.vector.memset(onesb[:], 1.0)
        eps64_t = const.tile([128, 1], f32, tag="eps64")
        nc.vector.memset(eps64_t[:], EPS * D)
        ln8_t = const.tile([128, 1], f32, tag="ln8")
        nc.vector.memset(ln8_t[:], LN8)

        # persistent tensors
        qkt = big.tile([65, 2, NHC, T], bf16, tag="qkt")
        vsb = big.tile([128, TCN, NHC, 65], bf16, tag="vsb")
        # augmented q~/k~ in token-major layout, input to the transposes:
        # [:, t, h, 0, :] = [q*rq/8 | a2+a3*mi], [:, t, h, 1, :] =
        # [k_raw_roped | mj*sqrt(ssk)/8]
        augP = big.tile([128, TCN, NHC, 2, 65], bf16, tag="augP")
        # per-key exp scale rkn = 8/sqrt(ssk) and its Schraudolph premult
        rkn = const.tile([128, TCN, NHC], f32, tag="rkn")
        askn = const.tile([128, TCN, NHC], f32, tag="askn")
        # yn rows 0-63: unnormalized head output; row 64: softmax denominator
        yn = [big.tile([65, T], bf16, name=f"yn{h}", tag=f"yn{h}")
              for h in range(NHC)]
        rcp = const.tile([128, TCN, NHC], f32, tag="rcp")
        yacc = big.tile([128, TCN, C], f32, tag="yacc")
        nc.vector.tensor_copy(
            out=vsb[:, :, :, 64], in_=onesb[:, :, None].to_broadcast([128, TCN, NHC])
        )
        # v-augment coefficients (a2 + a3*m) straight into augP's q column
        for h in range(NHC):
            nc.vector.tensor_scalar(
                out=augP[:, :, h, 0, 64], in0=mf[:],
                scalar1=cf[:, 2 * h + 1 : 2 * h + 2],
                scalar2=cf[:, 2 * h : 2 * h + 1],
                op0=OP.mult, op1=OP.add,
            )

        # ---- Phase A: QKV projection, rope, rms, augment, transpose ----
        with tc.tile_pool(name="psA", bufs=3, space="PSUM") as psA, \
             tc.tile_pool(name="psT", bufs=2, space="PSUM") as psT, \
             tc.tile_pool(name="scrA", bufs=6) as scrA:
            pq_l, ro_l, rt_l, rq_l, ssum_l, lnv_l = {}, {}, {}, {}, {}, {}
            raddq_l = {}

            def stage0(t):   # QKV projection (tensor)
                pq = psA.tile([128, 576], f32, tag="pq")
                pq_l[t] = pq
                tsl = slice(t * 128, (t + 1) * 128)
                for cc in range(3):
                    lhs = xt[:, cc, tsl]
                    nc.tensor.matmul(
                        pq[:, 0:512], lhs, wq[:, cc, 0:512],
                        start=(cc == 0), stop=(cc == 2),
                    )
                    nc.tensor.matmul(
                        pq[:, 512:576], lhs, wq[:, cc, 512:576],
                        start=(cc == 0), stop=(cc == 2),
                    )

            def stage1(t):   # rope mults (DVE) + v copy (scalar); frees pq
                pq = pq_l.pop(t)
                z5 = pq[:, 0:384].rearrange("p (hq hf d) -> p hq hf d",
                                            hq=6, hf=2)
                ro = scrA.tile([128, 6, 2, 32], bf16, tag="ro")
                rt = scrA.tile([128, 6, 2, 32], bf16, tag="rt")
                ro_l[t], rt_l[t] = ro, rt
                nc.vector.tensor_tensor(ro[:], z5, csF[:, t], OP.mult)
                zswap = bass.AP(
                    tensor=pq.tensor, offset=pq.offset + 32,
                    ap=[list(pq.ap[0])] + [[64, 6], [-32, 2], [1, 32]],
                )
                nc.vector.tensor_tensor(rt[:], zswap, snF[:, t], OP.mult)
                nc.scalar.activation(
                    out=vsb[:, t, :, 0:64],
                    in_=pq[:, 384:576].rearrange("p (h d) -> p h d", h=NHC),
                    func=AF.Copy,
                )

            def stage2(t):   # rope adds (gps): k-half lands raw in augP,
                ro, rt = ro_l.pop(t), rt_l.pop(t)   # q-half in scratch
                raddq = scrA.tile([128, NHC, 2, 32], bf16, tag="raddq")
                raddq_l[t] = raddq
                nc.gpsimd.tensor_tensor(
                    raddq[:], ro[:, 0::2], rt[:, 0::2], OP.add)
                ak = augP[:, t, :, 1, 0:64].rearrange(
                    "p h (hf d) -> p h hf d", hf=2)
                nc.gpsimd.tensor_tensor(ak, ro[:, 1::2], rt[:, 1::2], OP.add)

            def stage3(t):   # squares + reduces (DVE), ln (scalar)
                raddq = raddq_l[t]
                sqq = scrA.tile([128, NHC, 64], bf16, tag="sqq")
                sqk = scrA.tile([128, NHC, 64], bf16, tag="sqk")
                rq4 = raddq[:].rearrange("p h hf d -> p h (hf d)")
                ak = augP[:, t, :, 1, 0:64]
                nc.vector.tensor_tensor(sqq[:], rq4, rq4, OP.mult)
                nc.vector.tensor_tensor(sqk[:], ak, ak, OP.mult)
                ssum = scrA.tile([128, NHC, 2], f32, tag="ssum")
                ssum_l[t] = ssum
                nc.vector.tensor_reduce(out=ssum[:, :, 0], in_=sqq[:],
                                        axis=AX.X, op=OP.add)
                nc.vector.tensor_reduce(out=ssum[:, :, 1], in_=sqk[:],
                                        axis=AX.X, op=OP.add)
                lnv = scrA.tile([128, NHC, 2], f32, tag="lnv")
                lnv_l[t] = lnv
                nc.scalar.activation(
                    out=lnv[:], in_=ssum[:], func=AF.Ln, bias=eps64_t[:],
                )

            def stage4(t):   # scales: rq (scalar), rkn (scalar),
                lnv = lnv_l.pop(t)          # augk (DVE recip + gps mult)
                ssum_l.pop(t)
                rq = scrA.tile([128, NHC], f32, tag="rq")
                rq_l[t] = rq
                nc.scalar.activation(out=rq[:], in_=lnv[:, :, 0], func=AF.Exp,
                                     scale=-0.5)
                nc.scalar.activation(out=rkn[:, t, :], in_=lnv[:, :, 1],
                                     func=AF.Exp, scale=-0.5, bias=ln8_t[:])
                aks = scrA.tile([128, NHC], f32, tag="aks")
                nc.vector.reciprocal(out=aks[:], in_=rkn[:, t, :])
                nc.gpsimd.tensor_tensor(
                    augP[:, t, :, 1, 64], aks[:],
                    mfb[:, t, None].to_broadcast([128, NHC]), OP.mult)

            def stage5(t):   # q normalize into augP (DVE, per head)
                raddq = raddq_l.pop(t)
                rq = rq_l.pop(t)
                for h in range(NHC):
                    nc.vector.tensor_scalar(
                        out=augP[:, t, h, 0, 0:64],
                        in0=raddq[:, h].rearrange("p hf d -> p (hf d)"),
                        scalar1=rq[:, h, None], scalar2=None, op0=OP.mult,
                    )

            def stage6(t):   # transposes (tensor) + qkt copies (scalar/DVE)
                tsl = slice(t * 128, (t + 1) * 128)
                ptr = psT.tile([65, 2, NHC * 128], bf16, tag="pt")
                for qk in range(2):
                    for h in range(NHC):
                        nc.tensor.transpose(
                            out=ptr[:, qk, h * 128 : (h + 1) * 128],
                            in_=augP[:, t, h, qk, :], identity=ident[:],
                        )
                    src = ptr[:, qk, :].rearrange("d (h c) -> d h c", h=NHC)
                    if qk == 0:
                        nc.scalar.activation(
                            out=qkt[:, 0, :, tsl], in_=src, func=AF.Copy,
                        )
                    else:
                        nc.vector.tensor_copy(out=qkt[:, 1, :, tsl], in_=src)

            skew = [(stage0, 0), (stage1, 1), (stage2, 2), (stage3, 3),
                    (stage4, 4), (stage5, 5), (stage6, 8)]
            for i in range(TCN + 8):
                for fn, dist in skew:
                    t = i - dist
                    if 0 <= t < TCN:
                        fn(t)

        # Schraudolph per-key scale: A_EXP * rkn (single op)
        nc.vector.tensor_scalar(
            out=askn[:], in0=rkn[:], scalar1=A_EXP, scalar2=None, op0=OP.mult)

        # ---- Phase B: attention (scores -> exp -> PV) per head ----
        with tc.tile_pool(name="psS", bufs=2, space="PSUM") as psS, \
             tc.tile_pool(name="psY", bufs=1, space="PSUM") as psY, \
             tc.tile_pool(name="att", bufs=8) as attp:

            def epilogue(h):
                # denominator rows -> token-partition layout via tiny PE
                # transposes into a borrowed scores slot, then reciprocal.
                # bf16 PSUM writes must be 4B aligned -> pad a dummy lane.
                dt_ps = psS.tile([128, TCN, 2], bf16, tag="ps",
                                 name=f"dt{h}")
                for t in range(TCN):
                    nc.tensor.transpose(
                        out=dt_ps[:, t, 0, None],
                        in_=yn[h][64:65, t * 128 : (t + 1) * 128],
                        identity=ident[64:65, 64:65],
                    )
                nc.vector.reciprocal(out=rcp[:, :, h], in_=dt_ps[:, :, 0])

            for h in range(NHC):
                py = psY.tile([65, T], f32, tag="py")
                ats = [None] * TCN

                def emit_pv(j, h=h, py=py, ats=ats):
                    for n in range(4):
                        nc.tensor.matmul(
                            py[:, n * 512 : (n + 1) * 512], vsb[:, j, h, :],
                            ats[j][:, n * 512 : (n + 1) * 512],
                            start=(j == 0), stop=(j == TCN - 1),
                        )

                for j in range(TCN):
                    kblk = qkt[:, 1, h, j * 128 : (j + 1) * 128]
                    at = attp.tile([128, T], bf16, tag="at")
                    ats[j] = at
                    for half in range(2):
                        ps = psS.tile([128, 1024], f32, tag="ps")
                        for n2 in range(2):
                            n = half * 2 + n2
                            nc.tensor.matmul(
                                ps[:, n2 * 512 : (n2 + 1) * 512], kblk,
                                qkt[:, 0, h, n * 512 : (n + 1) * 512],
                                start=True, stop=True,
                            )
                        asl = slice(half * 1024, (half + 1) * 1024)
                        if (2 * j + half) % 16 not in (1, 4, 6, 9, 12):
                            # exact exp on ScalarE, per-key rms scale fused
                            nc.scalar.activation(
                                out=at[:, asl], in_=ps[:], func=AF.Exp,
                                scale=rkn[:, j, h, None],
                            )
                        else:
                            # Schraudolph exp on DVE: bf16 bits via int16
                            nc.vector.tensor_scalar(
                                out=at[:, asl].bitcast(i16), in0=ps[:],
                                scalar1=askn[:, j, h, None], scalar2=B_EXP,
                                op0=OP.mult, op1=OP.add,
                            )
                    if j >= 2:
                        emit_pv(j - 2)
                    if j == 2 and h > 0:
                        epilogue(h - 1)
                emit_pv(TCN - 2)
                emit_pv(TCN - 1)
                # parallel split copy frees the PSUM banks ~1us sooner
                nc.scalar.activation(out=yn[h][:, 0:1024], in_=py[:, 0:1024],
                                     func=AF.Copy)
                nc.vector.tensor_copy(out=yn[h][:, 1024:2048],
                                      in_=py[:, 1024:2048])

        # ---- Phase C: per-head projection, combine with 1/den, store ----
        # Heads 0/1 stream first (their yn + rcp are long ready, covering
        # head 2's py-copy window), then head 2's epilogue + projections.
        with tc.tile_pool(name="psC", bufs=6, space="PSUM") as psC, \
             tc.tile_pool(name="psD", bufs=1, space="PSUM") as psD:
            po2_l = {}
            for t in range(TCN):
                tsl = slice(t * 128, (t + 1) * 128)
                po0 = psC.tile([128, C], f32, name=f"po0_{t}", tag="po")
                po1 = psC.tile([128, C], f32, name=f"po1_{t}", tag="po")
                nc.tensor.matmul(po0[:], yn[0][0:64, tsl], wp[:, 0, :],
                                 start=True, stop=True)
                nc.tensor.matmul(po1[:], yn[1][0:64, tsl], wp[:, 1, :],
                                 start=True, stop=True)
                nc.scalar.activation(
                    out=yacc[:, t, :], in_=po0[:], func=AF.Copy,
                    scale=rcp[:, t, 0, None],
                )
                nc.vector.scalar_tensor_tensor(
                    out=yacc[:, t, :], in0=po1[:], scalar=rcp[:, t, 1, None],
                    in1=yacc[:, t, :], op0=OP.mult, op1=OP.add,
                )
                if t == 3:
                    # head 2 epilogue: py copy done by now; den transposes
                    dt2 = psD.tile([128, TCN, 2], bf16, tag="dt2")
                    for tt in range(TCN):
                        nc.tensor.transpose(
                            out=dt2[:, tt, 0, None],
                            in_=yn[2][64:65, tt * 128 : (tt + 1) * 128],
                            identity=ident[64:65, 64:65],
                        )
                    nc.vector.reciprocal(out=rcp[:, :, 2], in_=dt2[:, :, 0])
                if t >= 3:
                    for tp in ([t - 3] if t < TCN - 1 else
                               [t - 3, t - 2, t - 1, t]):
                        po2 = psC.tile([128, C], f32, name=f"po2_{tp}",
                                       tag="po")
                        po2_l[tp] = po2
                        nc.tensor.matmul(po2[:], yn[2][0:64,
                                         tp * 128 : (tp + 1) * 128],
                                         wp[:, 2, :], start=True, stop=True)
                        nc.vector.scalar_tensor_tensor(
                            out=yacc[:, tp, :], in0=po2[:],
                            scalar=rcp[:, tp, 2, None],
                            in1=yacc[:, tp, :], op0=OP.mult, op1=OP.add,
                        )
                        if tp % 4 == 3:
                            g0 = tp - 3
                            nc.sync.dma_start(
                                out=out[g0 * 128 : (g0 + 4) * 128, :]
                                .rearrange("(n p) c -> p n c", p=128),
                                in_=yacc[:, g0 : g0 + 4, :],
                            )

    _split_multi_waits(nc)
    return nc


_NC = None
LAST_RESULTS = None


def _get_nc():
    global _NC
    if _NC is None:
        _NC = _build_nc()
    return _NC


def kernel(x, cos, sin, token_is_mask, Wq, Wk, Wv, Wproj, mask_bias_raw,
           bias_scale, **_kw):
    bf = ml_dtypes.bfloat16
    x = np.asarray(x, np.float32)
    cos2 = np.asarray(cos, np.float32)[0, :, 0, :]                         # (T,32)
    sin2 = np.asarray(sin, np.float32)[0, :, 0, :]
    # partition-major rope table [128, TCN, 3, 32] = [cos | sin | -sin],
    # token t = n*128 + p
    rt3 = np.stack([cos2, sin2, -sin2], axis=1)                            # (T,3,32)
    ropet = np.ascontiguousarray(
        rt3.reshape(TCN, 128, 3, 32).transpose(1, 0, 2, 3))
    m = np.asarray(token_is_mask, np.int32)
    Wq = np.asarray(Wq, np.float32)
    Wk = np.asarray(Wk, np.float32)
    Wv = np.asarray(Wv, np.float32)
    Wp = np.asarray(Wproj, np.float32)
    g = (0.5 * np.tanh(np.asarray(mask_bias_raw, np.float64))
         * float(np.asarray(bias_scale))).astype(np.float32)  # (H,3)

    in_maps = []
    for core in range(8):
        b = core // 2
        hs = NHC * (core % 2)
        xTb = np.ascontiguousarray(x[b].T).astype(bf)          # (C,T)
        wqkv = np.zeros((C, 576), np.float32)
        wpt = np.zeros((NHC, D, C), np.float32)
        coefs = np.zeros((2 * NHC,), np.float32)
        for i in range(NHC):
            h = hs + i
            sl = slice(h * D, (h + 1) * D)
            wqkv[:, i * 128 + 0 : i * 128 + 64] = Wq[sl].T
            wqkv[:, i * 128 + 64 : i * 128 + 128] = Wk[sl].T
            wqkv[:, 384 + i * 64 : 384 + (i + 1) * 64] = Wv[sl].T
            wpt[i] = Wp[:, sl].T
            b01 = float(np.clip(g[h, 1], -2.0, 2.0))
            b10 = float(np.clip(g[h, 0], -2.0, 2.0))
            b11 = float(np.clip(g[h, 0] + g[h, 1] + g[h, 2], -2.0, 2.0))
            coefs[2 * i] = b01            # a2
            coefs[2 * i + 1] = b11 - b10 - b01  # a3
        in_maps.append(
            dict(
                xT=xTb,
                wqkv=wqkv.astype(bf),
                wpt=wpt.astype(bf),
                ropet=ropet,
                micf=np.concatenate(
                    [np.ascontiguousarray(m[b].reshape(TCN, 128).T)
                     .view(np.float32),
                     np.tile(coefs[None, :], (128, 1))], axis=1),
            )
        )

    nc = _get_nc()
    res = run_bass_kernel_spmd(nc, in_maps, list(range(8)))
    global LAST_RESULTS
    LAST_RESULTS = res
    out = np.zeros((B, T, C), np.float32)
    for b in range(B):
        out[b] = res.results[2 * b]["out"] + res.results[2 * b + 1]["out"]
    return out


# revision 9
# speedup vs baseline: 1.1902x; 1.1902x over previous
"""Multi-head attention (RoPE + RMS-norm + structured mask bias) on 8 trn2
NeuronCores.

Sharding: B=4 batches x 2 half-head groups -> 8 cores. Core c handles batch
c//2 and heads 3*(c%2) .. 3*(c%2)+2. Each core computes per-head partial
outputs y_h @ Wproj_h.T summed over its 3 heads; the host adds the two
half-head partials per batch.

Math notes:
- bias = clip(g1*mi + g2*mj + g3*mi*mj, +-2) with mi,mj in {0,1} takes 4
  values; decomposed as a0 + a1*mi + a2*mj + a3*mi*mj. The a0 + a1*mi part
  is constant along the softmax axis (keys) and cancels; only
  (a2 + a3*mi)*mj survives. It is folded into the QK^T matmul as one extra
  contraction dim.
- The K-side RMS scale rkn = 8/sqrt(ssk) is NOT applied to k. Instead the
  scores matmul computes ps = (q_rms/8).k_raw + augq*augk with
  augq = a2+a3*mi, augk = mj*sqrt(ssk)/8, and the exp applies rkn as a
  per-partition (per-key) ACT scale: exp(rkn*ps) == exp(q.k/8 + bias).
  This deletes the k-half of the phase-A normalize multiply.
- scores are in [-9.5, 9.5], so softmax runs without max subtraction.
  The denominator comes free from an appended ones-column on v.
- All matmuls run in bf16; PSUM stays f32. PE streams 1 moving col/cycle,
  so phase B (2*T^2/128 columns per head) is the tensor-time floor.
- rms scales via exp/ln -> ONE ACT table set (natural_log_exp_and_others).
- ~1/3 of exp tiles run on the otherwise-idle DVE as a Schraudolph
  approximation (bits = round(A*rkn*x + B) as int16 into the bf16 tile);
  the per-key A*rkn lives in a per-partition scalar AP.
- HAM warmup: identity matmuls during the initial DMA wait keep the PE
  clock at 2.4GHz (otherwise all of phase A runs throttled at 1.2GHz).
- Rope cos/sin are expanded once on-device into dense per-head tables so
  the per-chunk rope multiplies use fast dense access patterns instead of
  broadcast APs. Rope intermediates are bf16 (2x DVE throughput).
- Per-head denominator transposes + reciprocal are emitted two j-steps
  into the NEXT head's matmul stream (borrowed scores-pool PSUM slots) so
  the PE FIFO never stalls on the py->yn copy.
- Phase C: projection matmuls for heads 0/1 stream first (covering head
  2's epilogue), then head 2's; combine = ACT scale + two pipelined DVE
  scalar_tensor_tensor ops into a resident f32 accumulator; output DMA'd
  per 4-chunk group.
"""

from contextlib import ExitStack

import numpy as np
import ml_dtypes

import concourse.bass as bass
import concourse.tile as tile
from concourse import mybir
from concourse.masks import make_identity
from concourse.bass_utils import run_bass_kernel_spmd
from concourse.vector_clock import ScopedClock
import concourse.tile as tile_mod

B, T, C, H, D = 4, 2048, 384, 6, 64
NHC = 3           # heads per core
TCN = T // 128    # 16 token chunks
EPS = float(np.finfo(np.float32).eps)
LN8 = float(np.log(8.0))

# Schraudolph exp in bf16 bit space: bits = round(A*x + B)
LOG2E = 1.4426950408889634
A_EXP = 128.0 * LOG2E
B_EXP = 127.0 * 128.0 - 5.6

f32 = mybir.dt.float32
bf16 = mybir.dt.bfloat16
i16 = mybir.dt.int16
i32 = mybir.dt.int32
OP = mybir.AluOpType
AF = mybir.ActivationFunctionType
AX = mybir.AxisListType

# ---------------------------------------------------------------------------
# Workaround: this container's walrus accepts at most ONE sync wait per
# instruction. Split the TileContext final drain, and post-process all
# instructions, hoisting extra waits onto same-engine NoOps.
# ---------------------------------------------------------------------------
_ctr = [0]


def _drain_and_barrier_split(self, tick_clock, wait_clock):
    nc = self.nc
    drain_inst = nc.sync.drain()
    wait_clock.add_sem_waits(
        drain_inst.ins, ScopedClock({None: tick_clock.global_clock})
    )
    mi = drain_inst.ins
    si = mi.sync_info
    if si is not None and len(si.on_wait) > 1:
        waits = list(si.on_wait)
        mi.sync_info = mybir.SyncInfo(on_wait=waits[:1], on_update=list(si.on_update))
        for i in range(1, len(waits)):
            extra = nc.sync.drain()
            extra.ins.sync_info = mybir.SyncInfo(on_wait=[waits[i]], on_update=[])
    nc.all_engine_barrier()
    assert self.sems is not None
    popped = nc._tile_sem_poison_stack.pop()
    assert popped is self._sem_poison
    nc.clear_and_free_semaphores(list(self.sems.allocated().values()))
    nc.all_engine_barrier()


tile_mod.TileContext._drain_and_barrier = _drain_and_barrier_split


def _split_multi_waits(nc):
    for f in nc.m.functions:
        for bb in f.blocks:
            insts = bb.instructions
            out = []
            changed = False
            for inst in insts:
                si = getattr(inst, "sync_info", None)
                if si is not None and si.on_wait and len(si.on_wait) > 1:
                    waits = list(si.on_wait)
                    for w in waits[:-1]:
                        _ctr[0] += 1
                        out.append(
                            mybir.InstNoOp(
                                name=f"WSPLIT-{_ctr[0]}",
                                engine=inst.engine,
                                ins=[],
                                outs=[],
                                sync_info=mybir.SyncInfo(on_wait=[w], on_update=[]),
                            )
                        )
                    inst.sync_info = mybir.SyncInfo(
                        on_wait=[waits[-1]], on_update=list(si.on_update)
                    )
                    changed = True
                out.append(inst)
            if changed:
                insts[:] = out


# ---------------------------------------------------------------------------
# Device program (SPMD: same program on all 8 cores, data differs)
# ---------------------------------------------------------------------------


def _build_nc():
    nc = bass.Bass()
    # all tables partition-major so every DMA line is one contiguous burst
    xT = nc.declare_dram_parameter("xT", [C, T], bf16, isOutput=False)
    wqkv = nc.declare_dram_parameter("wqkv", [C, 576], bf16, isOutput=False)
    wpt = nc.declare_dram_parameter("wpt", [NHC, D, C], bf16, isOutput=False)
    # rope table: [...] = [cos | sin | -sin] packed per token chunk
    ropet = nc.declare_dram_parameter("ropet", [128, TCN, 3, 32], f32,
                                      isOutput=False)
    micf = nc.declare_dram_parameter("micf", [128, TCN + 2 * NHC], f32,
                                     isOutput=False)
    out = nc.declare_dram_parameter("out", [T, C], f32, isOutput=True)

    with tile.TileContext(nc) as tc, ExitStack() as ctx:
        const = ctx.enter_context(tc.tile_pool(name="const", bufs=1))
        big = ctx.enter_context(tc.tile_pool(name="big", bufs=1))

        # ---- loads: small consts first so phase A's elementwise work can
        # start as soon as the first QKV chunk lands ----
        xt = big.tile([128, 3, T], bf16, tag="xt")
        wq = big.tile([128, 3, 576], bf16, tag="wq")
        xTr = xT[:].rearrange("(n p) t -> p n t", p=128)
        nc.sync.dma_start(out=wq[:], in_=wqkv[:].rearrange("(n p) d -> p n d", p=128))
        nc.sync.dma_start(out=xt[:, 0, 0:512], in_=xTr[:, 0, 0:512])
        nc.sync.dma_start(out=xt[:, 1, 0:512], in_=xTr[:, 1, 0:512])
        nc.sync.dma_start(out=xt[:, 2, 0:512], in_=xTr[:, 2, 0:512])
        rtb = const.tile([128, TCN, 3, 32], f32, tag="rtb")
        nc.sync.dma_start(out=rtb[:], in_=ropet[:])
        micf_t = const.tile([128, TCN + 2 * NHC], f32, tag="micf")
        nc.sync.dma_start(out=micf_t[:], in_=micf[:])
        mi_t = micf_t[:, 0:TCN].bitcast(i32)
        cf = micf_t[:, TCN : TCN + 2 * NHC]
        nc.sync.dma_start(out=xt[:, 0, 512:T], in_=xTr[:, 0, 512:T])
        nc.sync.dma_start(out=xt[:, 1, 512:T], in_=xTr[:, 1, 512:T])
        nc.sync.dma_start(out=xt[:, 2, 512:T], in_=xTr[:, 2, 512:T])
        wp = const.tile([D, NHC, C], bf16, tag="wp")
        nc.sync.dma_start(out=wp[:], in_=wpt[:].rearrange("h d c -> d h c"))

        ident = const.tile([128, 128], bf16, tag="ident")
        make_identity(nc, ident[:])

        # ---- HAM warmup: dense identity matmuls while the DMAs land, so
        # the PE clock gate opens (K=8/8) before the real work starts ----
        with tc.tile_pool(name="psW", bufs=2, space="PSUM") as psW:
            for i in range(44):
                pw = psW.tile([128, 128], f32, tag="pw")
                nc.tensor.matmul(pw[:], ident[:], ident[:], start=True,
                                 stop=True)

        cs = rtb[:, :, 0, :]                     # [128, TCN, 32]
        sn2 = rtb[:, :, 1:3, :]                  # [128, TCN, 2, 32]

        mf = const.tile([128, TCN], f32, tag="mf")
        nc.vector.tensor_copy(out=mf[:], in_=mi_t)
        mfb = const.tile([128, TCN], bf16, tag="mfb")
        nc.gpsimd.tensor_copy(out=mfb[:], in_=mf[:])
        onesb = const.tile([128, TCN], bf16, tag="onesb")
        nc.vector.memset(onesb[:], 1.0)
        eps64_t = const.tile([128, 1], f32, tag="eps64")
        nc.vector.memset(eps64_t[:], EPS * D)
        ln8_t = const.tile([128, 1], f32, tag="ln8")
        nc.vector.memset(ln8_t[:], LN8)

        # persistent tensors
        qkt = big.tile([65, 2, NHC, T], bf16, tag="qkt")
        vsb = big.tile([128, TCN, NHC, 65], bf16, tag="vsb")
        # augmented q~/k~ in token-major layout, input to the transposes:
        # [:, t, h, 0, :] = [q*rq/8 | a2+a3*mi], [:, t, h, 1, :] =
        # [k_raw_roped | mj*sqrt(ssk)/8]
        augP = big.tile([128, TCN, NHC, 2, 65], bf16, tag="augP")
        # per-key exp scale rkn = 8/sqrt(ssk) and its Schraudolph premult
        rkn = const.tile([128, TCN, NHC], f32, tag="rkn")
        askn = const.tile([128, TCN, NHC], f32, tag="askn")
        # yn rows 0-63: unnormalized head output; row 64: softmax denominator
        yn = [big.tile([65, T], bf16, name=f"yn{h}", tag=f"yn{h}")
              for h in range(NHC)]
        rcp = const.tile([128, TCN, NHC], f32, tag="rcp")
        yacc = big.tile([128, TCN, C], f32, tag="yacc")
        nc.vector.tensor_copy(
            out=vsb[:, :, :, 64], in_=onesb[:, :, None].to_broadcast([128, TCN, NHC])
        )
        # v-augment coefficients (a2 + a3*m) straight into augP's q column
        for h in range(NHC):
            nc.vector.tensor_scalar(
                out=augP[:, :, h, 0, 64], in0=mf[:],
                scalar1=cf[:, 2 * h + 1 : 2 * h + 2],
                scalar2=cf[:, 2 * h : 2 * h + 1],
                op0=OP.mult, op1=OP.add,
            )

        # ---- Phase A: QKV projection, rope, rms, augment, transpose ----
        with tc.tile_pool(name="psA", bufs=3, space="PSUM") as psA, \
             tc.tile_pool(name="psT", bufs=2, space="PSUM") as psT, \
             tc.tile_pool(name="scrA", bufs=6) as scrA:
            pq_l, ro_l, rt_l, rq_l, lnv_l = {}, {}, {}, {}, {}

            def stage0(t):   # QKV projection (tensor)
                pq = psA.tile([128, 576], f32, tag="pq")
                pq_l[t] = pq
                tsl = slice(t * 128, (t + 1) * 128)
                for cc in range(3):
                    lhs = xt[:, cc, tsl]
                    nc.tensor.matmul(
                        pq[:, 0:512], lhs, wq[:, cc, 0:512],
                        start=(cc == 0), stop=(cc == 2),
                    )
                    nc.tensor.matmul(
                        pq[:, 512:576], lhs, wq[:, cc, 512:576],
                        start=(cc == 0), stop=(cc == 2),
                    )

            def stage1(t):   # rope mults (DVE) + v copy (scalar)
                pq = pq_l[t]
                z5 = pq[:, 0:384].rearrange("p (hq hf d) -> p hq hf d",
                                            hq=6, hf=2)
                csb = cs[:, t, None, None, :].to_broadcast([128, 6, 2, 32])
                snb = sn2[:, t, None, :, :].to_broadcast([128, 6, 2, 32])
                ro = scrA.tile([128, 6, 2, 32], bf16, tag="ro")
                rt = scrA.tile([128, 6, 2, 32], bf16, tag="rt")
                ro_l[t], rt_l[t] = ro, rt
                nc.vector.tensor_tensor(ro[:], z5, csb, OP.mult)
                zswap = bass.AP(
                    tensor=pq.tensor, offset=pq.offset + 32,
                    ap=[list(pq.ap[0])] + [[64, 6], [-32, 2], [1, 32]],
                )
                nc.vector.tensor_tensor(rt[:], zswap, snb, OP.mult)
                nc.scalar.activation(
                    out=vsb[:, t, :, 0:64],
                    in_=pq[:, 384:576].rearrange("p (h d) -> p h d", h=NHC),
                    func=AF.Copy,
                )

            def stage2(t):   # rms stats from PRE-rope q,k (rope preserves
                pq = pq_l.pop(t)   # norms): square (scalar) + reduce (DVE)
                sqz = scrA.tile([128, 6, 64], bf16, tag="sqz")
                nc.scalar.activation(
                    out=sqz[:],
                    in_=pq[:, 0:384].rearrange("p (hq d) -> p hq d", hq=6),
                    func=AF.Square,
                )
                ssum = scrA.tile([128, 6], f32, tag="ssum")
                nc.vector.tensor_reduce(out=ssum[:], in_=sqz[:], axis=AX.X,
                                        op=OP.add)
                lnv = scrA.tile([128, 6], f32, tag="lnv")
                lnv_l[t] = lnv
                nc.scalar.activation(
                    out=lnv[:], in_=ssum[:], func=AF.Ln, bias=eps64_t[:],
                )

            def stage3(t):   # rope add (gps) straight into augP (k raw)
                ro, rt = ro_l.pop(t), rt_l.pop(t)
                dst = augP[:, t, :, :, 0:64].rearrange(
                    "p h qk (hf d) -> p (h qk) hf d", hf=2)
                nc.gpsimd.tensor_tensor(dst, ro[:], rt[:], OP.add)

            def stage4(t):   # scales: rq/rkn (scalar exp), augk col
                lnv = lnv_l.pop(t)
                rq = scrA.tile([128, NHC], f32, tag="rq")
                rq_l[t] = rq
                nc.scalar.activation(out=rq[:], in_=lnv[:, 0::2], func=AF.Exp,
                                     scale=-0.5)
                nc.scalar.activation(out=rkn[:, t, :], in_=lnv[:, 1::2],
                                     func=AF.Exp, scale=-0.5, bias=ln8_t[:])
                aks = scrA.tile([128, NHC], f32, tag="aks")
                nc.vector.reciprocal(out=aks[:], in_=rkn[:, t, :])
                nc.gpsimd.tensor_tensor(
                    augP[:, t, :, 1, 64], aks[:],
                    mfb[:, t, None].to_broadcast([128, NHC]), OP.mult)

            def stage5(t):   # q normalize in-place in augP (gps, one op)
                rq = rq_l.pop(t)
                aq = augP[:, t, :, 0, 0:64]
                nc.gpsimd.tensor_tensor(
                    aq, aq, rq[:, :, None].to_broadcast([128, NHC, 64]),
                    OP.mult)

            def stage6(t):   # transposes (tensor) + qkt copies (scalar/DVE)
                tsl = slice(t * 128, (t + 1) * 128)
                ptr = psT.tile([65, 2, NHC * 128], bf16, tag="pt")
                for qk in range(2):
                    for h in range(NHC):
                        nc.tensor.transpose(
                            out=ptr[:, qk, h * 128 : (h + 1) * 128],
                            in_=augP[:, t, h, qk, :], identity=ident[:],
                        )
                src0 = ptr[:, 0, :].rearrange("d (h c) -> d h c", h=NHC)
                src1 = ptr[:, 1, :].rearrange("d (h c) -> d h c", h=NHC)
                nc.scalar.activation(out=qkt[:, 0, :, tsl], in_=src0,
                                     func=AF.Copy)
                nc.vector.tensor_copy(out=qkt[:, 1, :, tsl], in_=src1)

            skew = [(stage0, 0), (stage1, 1), (stage2, 2), (stage3, 3),
                    (stage4, 4), (stage5, 5), (stage6, 8)]
            for i in range(TCN + 8):
                for fn, dist in skew:
                    t = i - dist
                    if 0 <= t < TCN:
                        fn(t)

        # Schraudolph per-key scale: A_EXP * rkn (single op)
        nc.vector.tensor_scalar(
            out=askn[:], in0=rkn[:], scalar1=A_EXP, scalar2=None, op0=OP.mult)

        # ---- Phase B: attention (scores -> exp -> PV) per head ----
        with tc.tile_pool(name="psS", bufs=2, space="PSUM") as psS, \
             tc.tile_pool(name="psY", bufs=1, space="PSUM") as psY, \
             tc.tile_pool(name="att", bufs=8) as attp:

            def epilogue(h):
                # denominator rows -> token-partition layout via tiny PE
                # transposes into a borrowed scores slot, then reciprocal.
                # bf16 PSUM writes must be 4B aligned -> pad a dummy lane.
                dt_ps = psS.tile([128, TCN, 2], bf16, tag="ps",
                                 name=f"dt{h}")
                for t in range(TCN):
                    nc.tensor.transpose(
                        out=dt_ps[:, t, 0, None],
                        in_=yn[h][64:65, t * 128 : (t + 1) * 128],
                        identity=ident[64:65, 64:65],
                    )
                nc.vector.reciprocal(out=rcp[:, :, h], in_=dt_ps[:, :, 0])

            for h in range(NHC):
                py = psY.tile([65, T], f32, tag="py")
                ats = [None] * TCN

                def emit_pv(j, h=h, py=py, ats=ats):
                    for n in range(4):
                        nc.tensor.matmul(
                            py[:, n * 512 : (n + 1) * 512], vsb[:, j, h, :],
                            ats[j][:, n * 512 : (n + 1) * 512],
                            start=(j == 0), stop=(j == TCN - 1),
                        )

                for j in range(TCN):
                    kblk = qkt[:, 1, h, j * 128 : (j + 1) * 128]
                    at = attp.tile([128, T], bf16, tag="at")
                    ats[j] = at
                    for half in range(2):
                        ps = psS.tile([128, 1024], f32, tag="ps")
                        for n2 in range(2):
                            n = half * 2 + n2
                            nc.tensor.matmul(
                                ps[:, n2 * 512 : (n2 + 1) * 512], kblk,
                                qkt[:, 0, h, n * 512 : (n + 1) * 512],
                                start=True, stop=True,
                            )
                        asl = slice(half * 1024, (half + 1) * 1024)
                        if (2 * j + half) % 2 == 0:
                            # exact exp on ScalarE, per-key rms scale fused
                            nc.scalar.activation(
                                out=at[:, asl], in_=ps[:], func=AF.Exp,
                                scale=rkn[:, j, h, None],
                            )
                        else:
                            # Schraudolph exp on DVE: bf16 bits via int16
                            nc.vector.tensor_scalar(
                                out=at[:, asl].bitcast(i16), in0=ps[:],
                                scalar1=askn[:, j, h, None], scalar2=B_EXP,
                                op0=OP.mult, op1=OP.add,
                            )
                    if j >= 2:
                        emit_pv(j - 2)
                    if j == 2 and h > 0:
                        epilogue(h - 1)
                emit_pv(TCN - 2)
                emit_pv(TCN - 1)
                # parallel split copy frees the PSUM banks ~1us sooner
                nc.scalar.activation(out=yn[h][:, 0:1024], in_=py[:, 0:1024],
                                     func=AF.Copy)
                nc.vector.tensor_copy(out=yn[h][:, 1024:2048],
                                      in_=py[:, 1024:2048])

        # ---- Phase C: per-head projection, combine with 1/den, store ----
        # Heads 0/1 stream first (their yn + rcp are long ready, covering
        # head 2's py-copy window), then head 2's epilogue + projections.
        with tc.tile_pool(name="psC", bufs=6, space="PSUM") as psC, \
             tc.tile_pool(name="psD", bufs=1, space="PSUM") as psD:
            po2_l = {}
            for t in range(TCN):
                tsl = slice(t * 128, (t + 1) * 128)
                po0 = psC.tile([128, C], f32, name=f"po0_{t}", tag="po")
                po1 = psC.tile([128, C], f32, name=f"po1_{t}", tag="po")
                nc.tensor.matmul(po0[:], yn[0][0:64, tsl], wp[:, 0, :],
                                 start=True, stop=True)
                nc.tensor.matmul(po1[:], yn[1][0:64, tsl], wp[:, 1, :],
                                 start=True, stop=True)
                nc.scalar.activation(
                    out=yacc[:, t, :], in_=po0[:], func=AF.Copy,
                    scale=rcp[:, t, 0, None],
                )
                nc.vector.scalar_tensor_tensor(
                    out=yacc[:, t, :], in0=po1[:], scalar=rcp[:, t, 1, None],
                    in1=yacc[:, t, :], op0=OP.mult, op1=OP.add,
                )
                if t == 3:
                    # head 2 epilogue: py copy done by now; den transposes
                    dt2 = psD.tile([128, TCN, 2], bf16, tag="dt2")
                    for tt in range(TCN):
                        nc.tensor.transpose(
                            out=dt2[:, tt, 0, None],
                            in_=yn[2][64:65, tt * 128 : (tt + 1) * 128],
                            identity=ident[64:65, 64:65],
                        )
                    nc.vector.reciprocal(out=rcp[:, :, 2], in_=dt2[:, :, 0])
                if t >= 3:
                    for tp in ([t - 3] if t < TCN - 1 else
                               [t - 3, t - 2, t - 1, t]):
                        po2 = psC.tile([128, C], f32, name=f"po2_{tp}",
                                       tag="po")
                        po2_l[tp] = po2
                        nc.tensor.matmul(po2[:], yn[2][0:64,
                                         tp * 128 : (tp + 1) * 128],
                                         wp[:, 2, :], start=True, stop=True)
                        nc.vector.scalar_tensor_tensor(
                            out=yacc[:, tp, :], in0=po2[:],
                            scalar=rcp[:, tp, 2, None],
                            in1=yacc[:, tp, :], op0=OP.mult, op1=OP.add,
                        )
                        if tp % 4 == 3:
                            g0 = tp - 3
                            nc.sync.dma_start(
                                out=out[g0 * 128 : (g0 + 4) * 128, :]
                                .rearrange("(n p) c -> p n c", p=128),
                                in_=yacc[:, g0 : g0 + 4, :],
                            )

    _split_multi_waits(nc)
    return nc


_NC = None
LAST_RESULTS = None


def _get_nc():
    global _NC
    if _NC is None:
        _NC = _build_nc()
    return _NC


def kernel(x, cos, sin, token_is_mask, Wq, Wk, Wv, Wproj, mask_bias_raw,
           bias_scale, **_kw):
    bf = ml_dtypes.bfloat16
    x = np.asarray(x, np.float32)
    cos2 = np.asarray(cos, np.float32)[0, :, 0, :]                         # (T,32)
    sin2 = np.asarray(sin, np.float32)[0, :, 0, :]
    # partition-major rope table [128, TCN, 3, 32] = [cos | sin | -sin],
    # token t = n*128 + p
    rt3 = np.stack([cos2, sin2, -sin2], axis=1)                            # (T,3,32)
    ropet = np.ascontiguousarray(
        rt3.reshape(TCN, 128, 3, 32).transpose(1, 0, 2, 3))
    m = np.asarray(token_is_mask, np.int32)
    Wq = np.asarray(Wq, np.float32)
    Wk = np.asarray(Wk, np.float32)
    Wv = np.asarray(Wv, np.float32)
    Wp = np.asarray(Wproj, np.float32)
    g = (0.5 * np.tanh(np.asarray(mask_bias_raw, np.float64))
         * float(np.asarray(bias_scale))).astype(np.float32)  # (H,3)

    in_maps = []
    for core in range(8):
        b = core // 2
        hs = NHC * (core % 2)
        xTb = np.ascontiguousarray(x[b].T).astype(bf)          # (C,T)
        wqkv = np.zeros((C, 576), np.float32)
        wpt = np.zeros((NHC, D, C), np.float32)
        coefs = np.zeros((2 * NHC,), np.float32)
        for i in range(NHC):
            h = hs + i
            sl = slice(h * D, (h + 1) * D)
            wqkv[:, i * 128 + 0 : i * 128 + 64] = Wq[sl].T
            wqkv[:, i * 128 + 64 : i * 128 + 128] = Wk[sl].T
            wqkv[:, 384 + i * 64 : 384 + (i + 1) * 64] = Wv[sl].T
            wpt[i] = Wp[:, sl].T
            b01 = float(np.clip(g[h, 1], -2.0, 2.0))
            b10 = float(np.clip(g[h, 0], -2.0, 2.0))
            b11 = float(np.clip(g[h, 0] + g[h, 1] + g[h, 2], -2.0, 2.0))
            coefs[2 * i] = b01            # a2
            coefs[2 * i + 1] = b11 - b10 - b01  # a3
        in_maps.append(
            dict(
                xT=xTb,
                wqkv=wqkv.astype(bf),
                wpt=wpt.astype(bf),
                ropet=ropet,
                micf=np.concatenate(
                    [np.ascontiguousarray(m[b].reshape(TCN, 128).T)
                     .view(np.float32),
                     np.tile(coefs[None, :], (128, 1))], axis=1),
            )
        )

    nc = _get_nc()
    res = run_bass_kernel_spmd(nc, in_maps, list(range(8)))
    global LAST_RESULTS
    LAST_RESULTS = res
    out = np.zeros((B, T, C), np.float32)
    for b in range(B):
        out[b] = res.results[2 * b]["out"] + res.results[2 * b + 1]["out"]
    return out
